# revision 6
# baseline (speedup 1.0000x reference)
"""BalancedMoE (B=8192, D=2048, E=8, top-2) on 8 Trainium2 NeuronCores.

Strategy: expert-parallel with host-side sparse dispatch and K-SPLIT expert
pairing to balance the cores.

  - Host computes gate logits / top-2 routing / softmax gates and gathers
    each expert's tokens.  Each expert's GEMM is split into two k-halves
    (d < 1024 and d >= 1024).  Every core runs two jobs: slot A = one
    k-half of one of the 4 LARGEST experts (padded to the static C_A =
    max big count), slot B = one k-half of one of the 4 smallest (padded
    to C_B).  The two halves of an expert land on different cores; the
    host sums the two bf16 partials, adds the bias, and applies the gate
    weights.  Per-core PE work drops from 16*max(c_e) k-columns to
    8*(C_A + C_B) — within ~4% of perfect balance.
  - bf16 inputs run the PE at the same 1 column/cycle as fp32r but halve
    the DMA footprint, so the whole working set stays SBUF-resident.
  - ALL inputs ride ONE queue (Sync) in exact PE-consumption order, so
    FIFO order = priority and each transfer gets the full SDMA rate (the
    old two-queue split round-robined tokens against weights and the
    token front arrived ~2x late).  Outputs + job-B inputs ride the Act
    queue; the Act sequencer's head-of-line blocking on the startup
    output triggers naturally delays job-B loads until the job-A stream
    has drained.
  - Startup: the first k-tile's first 512 columns are a separate tiny DMA
    (~131 KB) so the first real matmul can start ~1.5 us after the queue
    opens; m0 is processed in k-singles, m1 in k-pairs, m2/m3 in k-quads
    (separate PSUM groups combined via f32 partials in SBUF) so the
    in-order PE queue chases the token arrival front; later rows run
    full-k accumulation.  A short PE warmup bridges the HAM cold window
    before the first operands land.
  - Job A's last row is processed at the very END (after job B) with
    per-j output DMAs, so the kernel tail is one 256-column drain + a
    65 KB store instead of a 478-column half-row.
"""

import os

import numpy as np

P = 128
B = 8192
D_LAT = 1024
D_EMB = 1024
D = D_LAT + D_EMB  # 2048
E = 8
TOPK = 2
N_CORES = 8
KT = D // P  # 16
KH = KT // 2  # k-tiles per half-job = 8
MT = D // P  # 16
NQ = 4  # startup m-rows of job A processed in k-chunks

# token k-groups in arrival order; group 0 is additionally split into
# (cols 0..512 | 512..CA) so the very first matmul starts early.  Each DMA
# trigger costs ~0.65 us on the issuing sequencer, so later groups are
# coarse.
TA_GROUPS = [(0, 1), (1, 2), (2, 4), (4, 8)]
# weight m-chunk DMA groups for job A, interleaved with the token groups on
# the Sync queue in consumption order.
WA_GROUPS = [(0, 1), (1, 2), (2, 4), (4, 16)]
WB_GROUPS = [(0, 8), (8, 16)]
TB_GROUPS = [(0, 8)]

N_WARM = 6  # PE warmup matmuls (bridge the HAM cold window)


# ----------------------------------------------------------------- device ---

_cache = {}


def _ntff_shim():
    """Register the axon NTFF profile hook that the boot skips when
    antenv.axon_hooks is missing (so BASS_TRACE=1 yields exec_time_ns)."""
    import sys
    import types

    if "antenv.axon_hooks" in sys.modules:
        return
    holder = [None]
    mod = types.ModuleType("antenv.axon_hooks")
    mod.set_axon_ntff_profile_hook = lambda h: holder.__setitem__(0, h)
    mod.get_axon_ntff_profile_hook = lambda: holder[0]
    sys.modules["antenv.axon_hooks"] = mod
    try:
        import antenv

        antenv.axon_hooks = mod
        from trn_agent_boot.trn_boot import _ntff_profile_via_ctypes

        mod.set_axon_ntff_profile_hook(
            _ntff_profile_via_ctypes("/opt/axon/libaxon_pjrt.so")
        )
    except Exception:
        pass


def _n_tiles(C):
    """Split C into moving-operand tiles of width 256..512 (>=256 columns per
    matmul keeps the PE at full rate; PSUM caps a tile at 512)."""
    assert C >= 512
    k = (C - 256) // 512 if C % 512 else C // 512
    rem = C - 512 * k
    sizes = [512] * k
    if rem == 0:
        pass
    elif rem <= 512:
        sizes.append(rem)
    else:  # 513..767: two tiles, both >= 256
        sizes.extend([rem - 256, 256])
    return sizes


def _build(CA, CB):
    import concourse.mybir as mybir
    from concourse import bacc
    from concourse.bass import ds
    from concourse.tile import TileContext

    dt = mybir.dt.bfloat16
    f32 = mybir.dt.float32

    def tiles_of(C):
        sizes = _n_tiles(C)
        offs = [0] * len(sizes)
        for j in range(1, len(sizes)):
            offs[j] = offs[j - 1] + sizes[j - 1]
        return sizes, offs

    a_sizes, a_offs = tiles_of(CA)
    b_sizes, b_offs = tiles_of(CB)

    nc = bacc.Bacc(
        "TRN2", target_bir_lowering=False, debug=False, num_devices=N_CORES
    )
    # w[ki, m, kl, o] = Whalf[m*128 + o, kl*128 + ki]  (partition-major)
    wa = nc.dram_tensor("wa", [P, MT, KH, P], dt, kind="ExternalInput")
    wb = nc.dram_tensor("wb", [P, MT, KH, P], dt, kind="ExternalInput")
    # t[ki, kl, c] = inp[idx[c], (h*8 + kl)*128 + ki]  (partition-major)
    ta = nc.dram_tensor("ta", [P, KH, CA], dt, kind="ExternalInput")
    tb = nc.dram_tensor("tb", [P, KH, CB], dt, kind="ExternalInput")
    outa = nc.dram_tensor("outa", [MT, P, CA], dt, kind="ExternalOutput")
    outb = nc.dram_tensor("outb", [MT, P, CB], dt, kind="ExternalOutput")

    nq = min(NQ, MT)

    with TileContext(nc) as tc:
        with (
            tc.tile_pool(name="w", bufs=1) as w_pool,
            tc.tile_pool(name="tok", bufs=1) as tok_pool,
            tc.tile_pool(name="acc", bufs=1) as acc_pool,
            tc.tile_pool(name="orow", bufs=6) as orow_pool,
            tc.tile_pool(name="warm", bufs=1) as warm_pool,
            tc.tile_pool(name="ps", bufs=8, space="PSUM") as ps_pool,
        ):
            # ---- tiles ----
            wa_tiles = [
                w_pool.tile([P, hi - lo, KH, P], dt, tag=f"wa{lo}",
                            name=f"wa{lo}")
                for lo, hi in WA_GROUPS
            ]
            wb_tiles = [
                w_pool.tile([P, hi - lo, KH, P], dt, tag=f"wb{lo}",
                            name=f"wb{lo}")
                for lo, hi in WB_GROUPS
            ]
            # token group 0 split: first 512 columns / rest
            ta0a = tok_pool.tile([P, 1, 512], dt, tag="ta0a", name="ta0a")
            ta0b = tok_pool.tile([P, 1, CA - 512], dt, tag="ta0b", name="ta0b")
            ta_tiles = [None] + [
                tok_pool.tile([P, hi - lo, CA], dt, tag=f"ta{lo}",
                              name=f"ta{lo}")
                for lo, hi in TA_GROUPS[1:]
            ]
            tb_tiles = [
                tok_pool.tile([P, hi - lo, CB], dt, tag=f"tb{lo}",
                              name=f"tb{lo}")
                for lo, hi in TB_GROUPS
            ]

            # ---- PE warmup (no input deps; bridges the HAM cold window
            # while the first tokens stream in).
            warm = warm_pool.tile([P, 512], dt)
            nc.gpsimd.memset(warm[:], 0)
            wps = ps_pool.tile([P, 512], f32, tag="ps")
            for i in range(N_WARM):
                nc.tensor.matmul(
                    wps, warm[:, :128], warm[:],
                    start=(i == 0), stop=(i == N_WARM - 1),
                )
            nc.vector.tensor_copy(warm[:], wps)

            # ---- input DMAs ----
            # Tokens ride the Sync queue in consumption order; the first
            # weight m-chunks ride the (initially idle) Act queue so neither
            # stream queues behind the other in the critical first ~10 us.
            nc.sync.dma_start(ta0a[:], ta.ap()[:, ds(0, 1), ds(0, 512)])
            nc.scalar.dma_start(wa_tiles[0][:], wa.ap()[:, ds(0, 1)])
            nc.sync.dma_start(ta0b[:], ta.ap()[:, ds(0, 1), ds(512, CA - 512)])
            nc.scalar.dma_start(wa_tiles[1][:], wa.ap()[:, ds(1, 1)])
            nc.sync.dma_start(ta_tiles[1][:], ta.ap()[:, ds(1, 1)])
            nc.scalar.dma_start(wa_tiles[2][:], wa.ap()[:, ds(2, 2)])
            nc.sync.dma_start(ta_tiles[2][:], ta.ap()[:, ds(2, 2)])
            nc.sync.dma_start(ta_tiles[3][:], ta.ap()[:, ds(4, 4)])
            nc.sync.dma_start(wa_tiles[3][:], wa.ap()[:, ds(4, 12)])

            def issue_b_inputs():
                # Act queue.  The Tile scheduler hoists dependency-free DMAs
                # to the front of the program (they'd steal the SDMA engines
                # from the critical token stream), so gate each job-B input
                # on the LAST job-A transfer via a 2-element dummy copy into
                # the destination tile (WAW forces the DMA to wait).
                gate = wa_tiles[3][:, 0, 0, ds(0, 2)]
                for gi, (lo, hi) in enumerate(WB_GROUPS):
                    nc.vector.tensor_copy(
                        wb_tiles[gi][:, 0, 0, ds(0, 2)], gate
                    )
                    nc.scalar.dma_start(
                        wb_tiles[gi][:], wb.ap()[:, ds(lo, hi - lo)]
                    )
                for gi, (lo, hi) in enumerate(TB_GROUPS):
                    nc.vector.tensor_copy(
                        tb_tiles[gi][:, 0, ds(0, 2)], gate
                    )
                    nc.scalar.dma_start(
                        tb_tiles[gi][:], tb.ap()[:, ds(lo, hi - lo)]
                    )

            def lhs(groups, tiles, m, k):
                for gi, (lo, hi) in enumerate(groups):
                    if m < hi:
                        return tiles[gi][:, m - lo, k, :]
                raise AssertionError

            def rhs_a(k, j):
                # job-A moving operand [P, a_sizes[j]] for k-tile k
                if k == 0:
                    if j == 0:
                        return ta0a[:, 0, :]
                    return ta0b[:, 0, ds(a_offs[j] - 512, a_sizes[j])]
                for gi, (lo, hi) in enumerate(TA_GROUPS):
                    if k < hi:
                        return ta_tiles[gi][:, k - lo, ds(a_offs[j], a_sizes[j])]
                raise AssertionError

            def rhs_b(k, j):
                for gi, (lo, hi) in enumerate(TB_GROUPS):
                    if k < hi:
                        return tb_tiles[gi][:, k - lo, ds(b_offs[j], b_sizes[j])]
                raise AssertionError

            # output row DMA in halves (Act queue): half A fires mid-row, so
            # only a short transfer trails the row's last drain
            def make_out_dmas(out_dram, sizes, offs, C):
                J = len(sizes)
                ja = 2 if J >= 3 else (1 if J >= 2 else 0)
                h_split = offs[ja] + sizes[ja]

                def dma_a(m, orow):
                    nc.scalar.dma_start(
                        out_dram.ap()[m][:, ds(0, h_split)],
                        orow[:, ds(0, h_split)],
                    )

                def dma_b(m, orow):
                    if C > h_split:
                        nc.scalar.dma_start(
                            out_dram.ap()[m][:, ds(h_split, C - h_split)],
                            orow[:, ds(h_split, C - h_split)],
                        )

                return ja, dma_a, dma_b

            a_ja, a_dma_a, a_dma_b = make_out_dmas(outa, a_sizes, a_offs, CA)
            b_ja, b_dma_a, b_dma_b = make_out_dmas(outb, b_sizes, b_offs, CB)
            J_A, J_B = len(a_sizes), len(b_sizes)

            # ---- job A startup: m0 in k-singles, m1 in k-pairs, m2/m3 in
            # k-quads, emitted in token/weight arrival order so the in-order
            # PE queue chases the DMA front.
            chunks = {
                0: [(0, 1), (1, 2), (2, 4), (4, 8)],
                1: [(0, 2), (2, 4), (4, 8)],
                2: [(0, 4), (4, 8)],
                3: [(0, 4), (4, 8)],
            }
            emit = [
                (0, 0), (0, 1), (1, 0), (0, 2), (1, 1),
                (2, 0), (3, 0), (0, 3), (1, 2), (2, 1), (3, 1),
            ]
            acc_tiles = {}
            orow_q = {}
            for m, qi in emit:
                klo, khi = chunks[m][qi]
                last = qi == len(chunks[m]) - 1
                for j in range(J_A):
                    psf = ps_pool.tile([P, 512], f32, tag="ps")
                    pj = psf[:, : a_sizes[j]]
                    for k in range(klo, khi):
                        nc.tensor.matmul(
                            pj,
                            lhs(WA_GROUPS, wa_tiles, m, k),
                            rhs_a(k, j),
                            start=(k == klo),
                            stop=(k == khi - 1),
                        )
                    if qi == 0:
                        a_full = acc_pool.tile([P, 512], f32, tag=f"acc{m}_{j}")
                        a = a_full[:, : a_sizes[j]]
                        acc_tiles[(m, j)] = a
                        # ACT engine: keeps DVE free during startup
                        nc.scalar.copy(a, pj)
                    elif not last:
                        a = acc_tiles[(m, j)]
                        nc.vector.tensor_add(a, a, pj)
                    else:
                        if m not in orow_q:
                            orow_q[m] = orow_pool.tile(
                                [P, CA], dt, tag="orow", name=f"orow_q{m}"
                            )
                        o = orow_q[m][:, ds(a_offs[j], a_sizes[j])]
                        nc.vector.tensor_add(o, acc_tiles[(m, j)], pj)
                        if j == a_ja:
                            a_dma_a(m, orow_q[m])
                if last:
                    a_dma_b(m, orow_q[m])

            issue_b_inputs()

            def steady_row(m, J, sizes, offs, ja, dma_a, dma_b, rhs, groups,
                           w_tiles, out_dram, C, nm, last_row=False):
                orow = orow_pool.tile([P, C], dt, tag="orow", name=nm)
                for j in range(J):
                    psf = ps_pool.tile([P, 512], f32, tag="ps")
                    pj = psf[:, : sizes[j]]
                    for k in range(KH):
                        nc.tensor.matmul(
                            pj,
                            lhs(groups, w_tiles, m, k),
                            rhs(k, j),
                            start=(k == 0),
                            stop=(k == KH - 1),
                        )
                    o = orow[:, ds(offs[j], sizes[j])]
                    nc.vector.tensor_copy(o, pj)
                    if last_row:
                        # per-j store: only a 256-col drain + 65 KB store
                        # trail the kernel's last matmul
                        nc.scalar.dma_start(
                            out_dram.ap()[m][:, ds(offs[j], sizes[j])], o
                        )
                    elif j == ja:
                        dma_a(m, orow)
                if not last_row:
                    dma_b(m, orow)

            # ---- job A steady rows (all but the last) ----
            for m in range(nq, MT - 1):
                steady_row(m, J_A, a_sizes, a_offs, a_ja, a_dma_a, a_dma_b,
                           rhs_a, WA_GROUPS, wa_tiles, outa, CA, f"oa{m}")

            # ---- job B rows (everything resident by now) ----
            for m in range(MT):
                steady_row(m, J_B, b_sizes, b_offs, b_ja, b_dma_a, b_dma_b,
                           rhs_b, WB_GROUPS, wb_tiles, outb, CB, f"ob{m}")

            # ---- job A last row LAST: its final j-tile is only 256 cols,
            # minimizing the drain+store tail after the last matmul.
            steady_row(MT - 1, J_A, a_sizes, a_offs, a_ja, a_dma_a, a_dma_b,
                       rhs_a, WA_GROUPS, wa_tiles, outa, CA, "oa_last",
                       last_row=True)
    nc.compile()
    return nc


def _get_program(CA, CB):
    key = (CA, CB)
    if key not in _cache:
        _cache[key] = _build(CA, CB)
    return _cache[key]


# ------------------------------------------------------------------- host ---


def kernel(x, y, W_experts, b_experts, W_gate, b_gate):
    import ml_dtypes

    bf16 = np.dtype(ml_dtypes.bfloat16)

    x = np.asarray(x, dtype=np.float32)
    y = np.asarray(y, dtype=np.float32)
    W_experts = np.asarray(W_experts, dtype=np.float32)
    b_experts = np.asarray(b_experts, dtype=np.float32)
    W_gate = np.asarray(W_gate, dtype=np.float32)
    b_gate = np.asarray(b_gate, dtype=np.float32)

    inp = np.concatenate([x, y], axis=1)  # [B, D]

    # ---- routing (host) ----
    logits = inp.astype(np.float64) @ W_gate.T.astype(np.float64) + b_gate
    order = np.argsort(-logits, axis=1, kind="stable")
    top2 = order[:, :TOPK]  # [B, 2]
    v = np.take_along_axis(logits, top2, axis=1)
    v = v - v.max(axis=1, keepdims=True)
    ev = np.exp(v)
    g = (ev / ev.sum(axis=1, keepdims=True)).astype(np.float32)  # [B, 2]

    counts = np.bincount(top2.ravel(), minlength=E)

    idx_list = []
    wgt_list = []
    for e in range(E):
        m0 = top2[:, 0] == e
        m1 = top2[:, 1] == e
        idx_e = np.concatenate([np.nonzero(m0)[0], np.nonzero(m1)[0]])
        w_e = np.concatenate([g[m0, 0], g[m1, 1]])
        idx_list.append(idx_e)
        wgt_list.append(w_e)

    # ---- k-split pairing: 4 biggest experts fill slot A, rest slot B ----
    by_size = np.argsort(-counts, kind="stable")
    big, small = by_size[:4], by_size[4:]
    CA = max(512, int(counts[big[0]]))
    CB = max(512, int(counts[small[0]]))
    # core 2*i   -> (big[i], half 0) + (small[i], half 0)
    # core 2*i+1 -> (big[i], half 1) + (small[i], half 1)
    slots = []  # per core: ((expertA, halfA), (expertB, halfB))
    for i in range(4):
        slots.append(((int(big[i]), 0), (int(small[i]), 0)))
        slots.append(((int(big[i]), 1), (int(small[i]), 1)))

    inp_bf = inp.astype(bf16)  # [B, D]
    w_r = W_experts.reshape(E, MT, P, KT, P)

    def w_half(e, h):
        # [P(ki), MT, KH, P(o)] bf16
        return np.ascontiguousarray(
            w_r[e][:, :, h * KH : (h + 1) * KH, :]
            .transpose(3, 0, 2, 1)
            .astype(bf16)
        )

    tok_cache = {}

    def tok_half(e, h, C):
        key = (e, h)
        if key not in tok_cache:
            sel = inp_bf[idx_list[e]].T.reshape(KT, P, -1)  # [KT, P, n_e]
            tok_cache[key] = sel[h * KH : (h + 1) * KH].transpose(1, 0, 2)
        n_e = len(idx_list[e])
        out = np.zeros((P, KH, C), dtype=bf16)
        out[:, :, :n_e] = tok_cache[key]
        return out

    in_maps = []
    for (ea, ha), (eb, hb) in slots:
        in_maps.append(
            {
                "wa": w_half(ea, ha),
                "ta": tok_half(ea, ha, CA),
                "wb": w_half(eb, hb),
                "tb": tok_half(eb, hb, CB),
            }
        )

    # ---- device ----
    if os.environ.get("BASS_TRACE"):
        _ntff_shim()
    from concourse.bass_utils import run_bass_kernel_spmd

    nc = _get_program(CA, CB)
    res = None
    for attempt in range(3):
        try:
            res = run_bass_kernel_spmd(nc, in_maps, core_ids=list(range(N_CORES)))
            break
        except Exception:
            # the axon-tunneled device occasionally reports a transient
            # NRT_EXEC_UNIT_UNRECOVERABLE; it recovers after a short wait
            if attempt == 2:
                raise
            import time

            time.sleep(20 * (attempt + 1))
            try:
                import jax

                jax.clear_caches()
            except Exception:
                pass
    globals()["_last_res"] = res
    if res.exec_time_ns is not None:
        print(f"HW exec time: {res.exec_time_ns} ns")

    # ---- combine (host): sum the two k-half partials, add bias, apply
    # gate weights, scatter.
    part = {}  # (expert, half) -> [n_e, D] f32
    for core, ((ea, ha), (eb, hb)) in enumerate(slots):
        n_a = len(idx_list[ea])
        part[(ea, ha)] = (
            res.results[core]["outa"].reshape(D, CA)[:, :n_a].T.astype(np.float32)
        )
        n_b = len(idx_list[eb])
        part[(eb, hb)] = (
            res.results[core]["outb"].reshape(D, CB)[:, :n_b].T.astype(np.float32)
        )

    fused = np.zeros((B, D), dtype=np.float32)
    for e in range(E):
        n_e = len(idx_list[e])
        if n_e == 0:
            continue
        rows = part[(e, 0)] + part[(e, 1)] + b_experts[e]
        fused[idx_list[e]] += rows * wgt_list[e][:, None]
    return fused


# revision 7
# speedup vs baseline: 1.0705x; 1.0705x over previous
"""BalancedMoE (B=8192, D=2048, E=8, top-2) on 8 Trainium2 NeuronCores.

Strategy: expert-parallel with host-side sparse dispatch and K-SPLIT expert
pairing to balance the cores.

  - Host computes gate logits / top-2 routing / softmax gates and gathers
    each expert's tokens.  Each expert's GEMM is split into two k-halves
    (d < 1024 and d >= 1024).  Every core runs two jobs: slot A = one
    k-half of one of the 4 LARGEST experts (padded to the static C_A =
    max big count), slot B = one k-half of one of the 4 smallest (padded
    to C_B).  The two halves of an expert land on different cores; the
    host sums the two bf16 partials, adds the bias, and applies the gate
    weights.  Per-core PE work drops from 16*max(c_e) k-columns to
    8*(C_A + C_B) — within ~4% of perfect balance.
  - bf16 inputs run the PE at the same 1 column/cycle as fp32r but halve
    the DMA footprint, so the whole working set stays SBUF-resident.
  - ALL inputs ride ONE queue (Sync) in exact PE-consumption order, so
    FIFO order = priority and each transfer gets the full SDMA rate (the
    old two-queue split round-robined tokens against weights and the
    token front arrived ~2x late).  Outputs + job-B inputs ride the Act
    queue; the Act sequencer's head-of-line blocking on the startup
    output triggers naturally delays job-B loads until the job-A stream
    has drained.
  - Startup: the first k-tile's first 512 columns are a separate tiny DMA
    (~131 KB) so the first real matmul can start ~1.5 us after the queue
    opens; m0 is processed in k-singles, m1 in k-pairs, m2/m3 in k-quads
    (separate PSUM groups combined via f32 partials in SBUF) so the
    in-order PE queue chases the token arrival front; later rows run
    full-k accumulation.  A short PE warmup bridges the HAM cold window
    before the first operands land.
  - Job A's last row is processed at the very END (after job B) with
    per-j output DMAs, so the kernel tail is one 256-column drain + a
    65 KB store instead of a 478-column half-row.
"""

import os

import numpy as np

P = 128
B = 8192
D_LAT = 1024
D_EMB = 1024
D = D_LAT + D_EMB  # 2048
E = 8
TOPK = 2
N_CORES = 8
KT = D // P  # 16
KH = KT // 2  # k-tiles per half-job = 8
MT = D // P  # 16
NQ = 4  # startup m-rows of job A processed in k-chunks

# token k-groups in arrival order; group 0 is additionally split into
# (cols 0..512 | 512..CA) so the very first matmul starts early.  Each DMA
# trigger costs ~0.65 us on the issuing sequencer, so later groups are
# coarse.
TA_GROUPS = [(0, 1), (1, 2), (2, 4), (4, 8)]
# weight m-chunk DMA groups for job A, interleaved with the token groups on
# the Sync queue in consumption order.
WA_GROUPS = [(0, 1), (1, 2), (2, 4), (4, 16)]
WB_GROUPS = [(0, 8), (8, 16)]
TB_GROUPS = [(0, 8)]

N_WARM = 4  # PE warmup matmuls (bridge the HAM cold window)


# ----------------------------------------------------------------- device ---

_cache = {}


def _ntff_shim():
    """Register the axon NTFF profile hook that the boot skips when
    antenv.axon_hooks is missing (so BASS_TRACE=1 yields exec_time_ns)."""
    import sys
    import types

    if "antenv.axon_hooks" in sys.modules:
        return
    holder = [None]
    mod = types.ModuleType("antenv.axon_hooks")
    mod.set_axon_ntff_profile_hook = lambda h: holder.__setitem__(0, h)
    mod.get_axon_ntff_profile_hook = lambda: holder[0]
    sys.modules["antenv.axon_hooks"] = mod
    try:
        import antenv

        antenv.axon_hooks = mod
        from trn_agent_boot.trn_boot import _ntff_profile_via_ctypes

        mod.set_axon_ntff_profile_hook(
            _ntff_profile_via_ctypes("/opt/axon/libaxon_pjrt.so")
        )
    except Exception:
        pass


def _n_tiles(C):
    """Split C into moving-operand tiles of width 256..512 (>=256 columns per
    matmul keeps the PE at full rate; PSUM caps a tile at 512)."""
    assert C >= 512
    k = (C - 256) // 512 if C % 512 else C // 512
    rem = C - 512 * k
    sizes = [512] * k
    if rem == 0:
        pass
    elif rem <= 512:
        sizes.append(rem)
    else:  # 513..767: two tiles, both >= 256
        sizes.extend([rem - 256, 256])
    return sizes


def _build(CA, CB):
    import concourse.mybir as mybir
    from concourse import bacc
    from concourse.bass import ds
    from concourse.tile import TileContext

    dt = mybir.dt.bfloat16
    f32 = mybir.dt.float32

    def tiles_of(C):
        sizes = _n_tiles(C)
        offs = [0] * len(sizes)
        for j in range(1, len(sizes)):
            offs[j] = offs[j - 1] + sizes[j - 1]
        return sizes, offs

    a_sizes, a_offs = tiles_of(CA)
    b_sizes, b_offs = tiles_of(CB)

    nc = bacc.Bacc(
        "TRN2", target_bir_lowering=False, debug=False, num_devices=N_CORES
    )
    # w[ki, m, kl, o] = Whalf[m*128 + o, kl*128 + ki]  (partition-major)
    wa = nc.dram_tensor("wa", [P, MT, KH, P], dt, kind="ExternalInput")
    wb = nc.dram_tensor("wb", [P, MT, KH, P], dt, kind="ExternalInput")
    # t[ki, kl, c] = inp[idx[c], (h*8 + kl)*128 + ki]  (partition-major)
    ta = nc.dram_tensor("ta", [P, KH, CA], dt, kind="ExternalInput")
    tb = nc.dram_tensor("tb", [P, KH, CB], dt, kind="ExternalInput")
    outa = nc.dram_tensor("outa", [MT, P, CA], dt, kind="ExternalOutput")
    outb = nc.dram_tensor("outb", [MT, P, CB], dt, kind="ExternalOutput")

    nq = min(NQ, MT)

    with TileContext(nc) as tc:
        with (
            tc.tile_pool(name="w", bufs=1) as w_pool,
            tc.tile_pool(name="tok", bufs=1) as tok_pool,
            tc.tile_pool(name="acc", bufs=1) as acc_pool,
            tc.tile_pool(name="orow", bufs=6) as orow_pool,
            tc.tile_pool(name="warm", bufs=1) as warm_pool,
            tc.tile_pool(name="ps", bufs=8, space="PSUM") as ps_pool,
        ):
            # ---- tiles ----
            wa_tiles = [
                w_pool.tile([P, hi - lo, KH, P], dt, tag=f"wa{lo}",
                            name=f"wa{lo}")
                for lo, hi in WA_GROUPS
            ]
            wb_tiles = [
                w_pool.tile([P, hi - lo, KH, P], dt, tag=f"wb{lo}",
                            name=f"wb{lo}")
                for lo, hi in WB_GROUPS
            ]
            # token group 0 split: first 512 columns / rest
            ta0a = tok_pool.tile([P, 1, 512], dt, tag="ta0a", name="ta0a")
            ta0b = tok_pool.tile([P, 1, CA - 512], dt, tag="ta0b", name="ta0b")
            ta_tiles = [None] + [
                tok_pool.tile([P, hi - lo, CA], dt, tag=f"ta{lo}",
                              name=f"ta{lo}")
                for lo, hi in TA_GROUPS[1:]
            ]
            tb_tiles = [
                tok_pool.tile([P, hi - lo, CB], dt, tag=f"tb{lo}",
                              name=f"tb{lo}")
                for lo, hi in TB_GROUPS
            ]

            # ---- PE warmup (no input deps; bridges the HAM cold window
            # while the first tokens stream in).
            warm = warm_pool.tile([P, 512], dt)
            nc.gpsimd.memset(warm[:], 0)
            wps = ps_pool.tile([P, 512], f32, tag="ps")
            for i in range(N_WARM):
                nc.tensor.matmul(
                    wps, warm[:, :128], warm[:],
                    start=(i == 0), stop=(i == N_WARM - 1),
                )
            nc.vector.tensor_copy(warm[:], wps)

            # ---- input DMAs ----
            # Tokens ride the Sync queue in consumption order; the first
            # weight m-chunks ride the (initially idle) Act queue so neither
            # stream queues behind the other in the critical first ~10 us.
            nc.sync.dma_start(ta0a[:], ta.ap()[:, ds(0, 1), ds(0, 512)])
            nc.scalar.dma_start(wa_tiles[0][:], wa.ap()[:, ds(0, 1)])
            nc.sync.dma_start(ta0b[:], ta.ap()[:, ds(0, 1), ds(512, CA - 512)])
            nc.scalar.dma_start(wa_tiles[1][:], wa.ap()[:, ds(1, 1)])
            nc.sync.dma_start(ta_tiles[1][:], ta.ap()[:, ds(1, 1)])
            nc.scalar.dma_start(wa_tiles[2][:], wa.ap()[:, ds(2, 2)])
            nc.sync.dma_start(ta_tiles[2][:], ta.ap()[:, ds(2, 2)])
            nc.sync.dma_start(ta_tiles[3][:], ta.ap()[:, ds(4, 4)])
            nc.sync.dma_start(wa_tiles[3][:], wa.ap()[:, ds(4, 12)])

            def issue_b_inputs():
                # Act queue.  The Tile scheduler hoists dependency-free DMAs
                # to the front of the program (they'd steal the SDMA engines
                # from the critical token stream), so gate each job-B input
                # on the LAST job-A transfer via a 2-element dummy copy into
                # the destination tile (WAW forces the DMA to wait).
                gate = wa_tiles[3][:, 0, 0, ds(0, 2)]
                for gi, (lo, hi) in enumerate(WB_GROUPS):
                    nc.vector.tensor_copy(
                        wb_tiles[gi][:, 0, 0, ds(0, 2)], gate
                    )
                    nc.scalar.dma_start(
                        wb_tiles[gi][:], wb.ap()[:, ds(lo, hi - lo)]
                    )
                for gi, (lo, hi) in enumerate(TB_GROUPS):
                    nc.vector.tensor_copy(
                        tb_tiles[gi][:, 0, ds(0, 2)], gate
                    )
                    nc.scalar.dma_start(
                        tb_tiles[gi][:], tb.ap()[:, ds(lo, hi - lo)]
                    )

            def lhs(groups, tiles, m, k):
                for gi, (lo, hi) in enumerate(groups):
                    if m < hi:
                        return tiles[gi][:, m - lo, k, :]
                raise AssertionError

            def rhs_a(k, j):
                # job-A moving operand [P, a_sizes[j]] for k-tile k
                if k == 0:
                    if j == 0:
                        return ta0a[:, 0, :]
                    return ta0b[:, 0, ds(a_offs[j] - 512, a_sizes[j])]
                for gi, (lo, hi) in enumerate(TA_GROUPS):
                    if k < hi:
                        return ta_tiles[gi][:, k - lo, ds(a_offs[j], a_sizes[j])]
                raise AssertionError

            def rhs_b(k, j):
                for gi, (lo, hi) in enumerate(TB_GROUPS):
                    if k < hi:
                        return tb_tiles[gi][:, k - lo, ds(b_offs[j], b_sizes[j])]
                raise AssertionError

            # output row DMA in halves (Act queue): half A fires mid-row, so
            # only a short transfer trails the row's last drain
            def make_out_dmas(out_dram, sizes, offs, C):
                J = len(sizes)
                ja = 2 if J >= 3 else (1 if J >= 2 else 0)
                h_split = offs[ja] + sizes[ja]

                def dma_a(m, orow):
                    nc.scalar.dma_start(
                        out_dram.ap()[m][:, ds(0, h_split)],
                        orow[:, ds(0, h_split)],
                    )

                def dma_b(m, orow):
                    if C > h_split:
                        nc.scalar.dma_start(
                            out_dram.ap()[m][:, ds(h_split, C - h_split)],
                            orow[:, ds(h_split, C - h_split)],
                        )

                return ja, dma_a, dma_b

            a_ja, a_dma_a, a_dma_b = make_out_dmas(outa, a_sizes, a_offs, CA)
            b_ja, b_dma_a, b_dma_b = make_out_dmas(outb, b_sizes, b_offs, CB)
            J_A, J_B = len(a_sizes), len(b_sizes)

            # ---- job A startup: m0 in k-singles, m1 in k-pairs, m2/m3 in
            # k-quads, emitted in token/weight arrival order so the in-order
            # PE queue chases the DMA front.
            chunks = {
                0: [(0, 1), (1, 2), (2, 4), (4, 8)],
                1: [(0, 2), (2, 4), (4, 8)],
                2: [(0, 4), (4, 8)],
                3: [(0, 4), (4, 8)],
            }
            emit = [
                (0, 0), (0, 1), (1, 0), (0, 2), (1, 1),
                (2, 0), (3, 0), (0, 3), (1, 2), (2, 1), (3, 1),
            ]
            acc_tiles = {}
            orow_q = {}
            for m, qi in emit:
                klo, khi = chunks[m][qi]
                last = qi == len(chunks[m]) - 1
                for j in range(J_A):
                    psf = ps_pool.tile([P, 512], f32, tag="ps")
                    pj = psf[:, : a_sizes[j]]
                    for k in range(klo, khi):
                        nc.tensor.matmul(
                            pj,
                            lhs(WA_GROUPS, wa_tiles, m, k),
                            rhs_a(k, j),
                            start=(k == klo),
                            stop=(k == khi - 1),
                        )
                    if qi == 0:
                        a_full = acc_pool.tile([P, 512], f32, tag=f"acc{m}_{j}")
                        a = a_full[:, : a_sizes[j]]
                        acc_tiles[(m, j)] = a
                        # ACT engine: keeps DVE free during startup
                        nc.scalar.copy(a, pj)
                    elif not last:
                        a = acc_tiles[(m, j)]
                        nc.vector.tensor_add(a, a, pj)
                    else:
                        if m not in orow_q:
                            orow_q[m] = orow_pool.tile(
                                [P, CA], dt, tag="orow", name=f"orow_q{m}"
                            )
                        o = orow_q[m][:, ds(a_offs[j], a_sizes[j])]
                        nc.vector.tensor_add(o, acc_tiles[(m, j)], pj)
                        if j == a_ja:
                            a_dma_a(m, orow_q[m])
                if last:
                    a_dma_b(m, orow_q[m])

            issue_b_inputs()

            def steady_row(m, J, sizes, offs, ja, dma_a, dma_b, rhs, groups,
                           w_tiles, out_dram, C, nm, last_row=False):
                orow = orow_pool.tile([P, C], dt, tag="orow", name=nm)
                for j in range(J):
                    psf = ps_pool.tile([P, 512], f32, tag="ps")
                    pj = psf[:, : sizes[j]]
                    for k in range(KH):
                        nc.tensor.matmul(
                            pj,
                            lhs(groups, w_tiles, m, k),
                            rhs(k, j),
                            start=(k == 0),
                            stop=(k == KH - 1),
                        )
                    o = orow[:, ds(offs[j], sizes[j])]
                    nc.vector.tensor_copy(o, pj)
                    if last_row:
                        # per-j store: only a 256-col drain + 65 KB store
                        # trail the kernel's last matmul
                        nc.scalar.dma_start(
                            out_dram.ap()[m][:, ds(offs[j], sizes[j])], o
                        )
                    elif j == ja:
                        dma_a(m, orow)
                if not last_row:
                    dma_b(m, orow)

            # ---- job A steady rows (all but the last) ----
            for m in range(nq, MT - 1):
                steady_row(m, J_A, a_sizes, a_offs, a_ja, a_dma_a, a_dma_b,
                           rhs_a, WA_GROUPS, wa_tiles, outa, CA, f"oa{m}")

            # ---- job B rows (everything resident by now) ----
            for m in range(MT):
                steady_row(m, J_B, b_sizes, b_offs, b_ja, b_dma_a, b_dma_b,
                           rhs_b, WB_GROUPS, wb_tiles, outb, CB, f"ob{m}")

            # ---- job A last row LAST: its final j-tile is only 256 cols,
            # minimizing the drain+store tail after the last matmul.
            steady_row(MT - 1, J_A, a_sizes, a_offs, a_ja, a_dma_a, a_dma_b,
                       rhs_a, WA_GROUPS, wa_tiles, outa, CA, "oa_last",
                       last_row=True)
    nc.compile()
    return nc


def _get_program(CA, CB):
    key = (CA, CB)
    if key not in _cache:
        _cache[key] = _build(CA, CB)
    return _cache[key]


# ------------------------------------------------------------------- host ---


def kernel(x, y, W_experts, b_experts, W_gate, b_gate):
    import ml_dtypes

    bf16 = np.dtype(ml_dtypes.bfloat16)

    x = np.asarray(x, dtype=np.float32)
    y = np.asarray(y, dtype=np.float32)
    W_experts = np.asarray(W_experts, dtype=np.float32)
    b_experts = np.asarray(b_experts, dtype=np.float32)
    W_gate = np.asarray(W_gate, dtype=np.float32)
    b_gate = np.asarray(b_gate, dtype=np.float32)

    inp = np.concatenate([x, y], axis=1)  # [B, D]

    # ---- routing (host) ----
    logits = inp.astype(np.float64) @ W_gate.T.astype(np.float64) + b_gate
    order = np.argsort(-logits, axis=1, kind="stable")
    top2 = order[:, :TOPK]  # [B, 2]
    v = np.take_along_axis(logits, top2, axis=1)
    v = v - v.max(axis=1, keepdims=True)
    ev = np.exp(v)
    g = (ev / ev.sum(axis=1, keepdims=True)).astype(np.float32)  # [B, 2]

    counts = np.bincount(top2.ravel(), minlength=E)

    idx_list = []
    wgt_list = []
    for e in range(E):
        m0 = top2[:, 0] == e
        m1 = top2[:, 1] == e
        idx_e = np.concatenate([np.nonzero(m0)[0], np.nonzero(m1)[0]])
        w_e = np.concatenate([g[m0, 0], g[m1, 1]])
        idx_list.append(idx_e)
        wgt_list.append(w_e)

    # ---- k-split pairing: 4 biggest experts fill slot A, rest slot B ----
    by_size = np.argsort(-counts, kind="stable")
    big, small = by_size[:4], by_size[4:]
    CA = max(512, int(counts[big[0]]))
    CB = max(512, int(counts[small[0]]))
    # core 2*i   -> (big[i], half 0) + (small[i], half 0)
    # core 2*i+1 -> (big[i], half 1) + (small[i], half 1)
    slots = []  # per core: ((expertA, halfA), (expertB, halfB))
    for i in range(4):
        slots.append(((int(big[i]), 0), (int(small[i]), 0)))
        slots.append(((int(big[i]), 1), (int(small[i]), 1)))

    inp_bf = inp.astype(bf16)  # [B, D]
    w_r = W_experts.reshape(E, MT, P, KT, P)

    def w_half(e, h):
        # [P(ki), MT, KH, P(o)] bf16
        return np.ascontiguousarray(
            w_r[e][:, :, h * KH : (h + 1) * KH, :]
            .transpose(3, 0, 2, 1)
            .astype(bf16)
        )

    tok_cache = {}

    def tok_half(e, h, C):
        key = (e, h)
        if key not in tok_cache:
            sel = inp_bf[idx_list[e]].T.reshape(KT, P, -1)  # [KT, P, n_e]
            tok_cache[key] = sel[h * KH : (h + 1) * KH].transpose(1, 0, 2)
        n_e = len(idx_list[e])
        out = np.zeros((P, KH, C), dtype=bf16)
        out[:, :, :n_e] = tok_cache[key]
        return out

    in_maps = []
    for (ea, ha), (eb, hb) in slots:
        in_maps.append(
            {
                "wa": w_half(ea, ha),
                "ta": tok_half(ea, ha, CA),
                "wb": w_half(eb, hb),
                "tb": tok_half(eb, hb, CB),
            }
        )

    # ---- device ----
    if os.environ.get("BASS_TRACE"):
        _ntff_shim()
    from concourse.bass_utils import run_bass_kernel_spmd

    nc = _get_program(CA, CB)
    res = None
    for attempt in range(3):
        try:
            res = run_bass_kernel_spmd(nc, in_maps, core_ids=list(range(N_CORES)))
            break
        except Exception:
            # the axon-tunneled device occasionally reports a transient
            # NRT_EXEC_UNIT_UNRECOVERABLE; it recovers after a short wait
            if attempt == 2:
                raise
            import time

            time.sleep(20 * (attempt + 1))
            try:
                import jax

                jax.clear_caches()
            except Exception:
                pass
    globals()["_last_res"] = res
    if res.exec_time_ns is not None:
        print(f"HW exec time: {res.exec_time_ns} ns")

    # ---- combine (host): sum the two k-half partials, add bias, apply
    # gate weights, scatter.
    part = {}  # (expert, half) -> [n_e, D] f32
    for core, ((ea, ha), (eb, hb)) in enumerate(slots):
        n_a = len(idx_list[ea])
        part[(ea, ha)] = (
            res.results[core]["outa"].reshape(D, CA)[:, :n_a].T.astype(np.float32)
        )
        n_b = len(idx_list[eb])
        part[(eb, hb)] = (
            res.results[core]["outb"].reshape(D, CB)[:, :n_b].T.astype(np.float32)
        )

    fused = np.zeros((B, D), dtype=np.float32)
    for e in range(E):
        n_e = len(idx_list[e])
        if n_e == 0:
            continue
        rows = part[(e, 0)] + part[(e, 1)] + b_experts[e]
        fused[idx_list[e]] += rows * wgt_list[e][:, None]
    return fused


# revision 8
# speedup vs baseline: 1.0747x; 1.0038x over previous
"""BalancedMoE (B=8192, D=2048, E=8, top-2) on 8 Trainium2 NeuronCores.

Q=4 variant: each expert's GEMM is split into FOUR k-quarters (512
features each).  The 32 (expert, quarter) jobs are assigned to 8 cores x 4
slots; slot s holds quarters of the two experts ranked (2s+1, 2s+2) by
token count, so the static slot capacities are the pairwise maxima
[2234, 2081, 2014, 1992] = 8321 columns/core vs 8496 for the k-half
scheme (and 8192 ideal).  The host sums the four bf16 partials, adds the
bias, applies the gate weights, and scatters.

DMA/startup strategy: slot-0 tokens ride the Sync queue in exact
PE-consumption order (fine-grained first chunks, then coarse) and the
first slot-0 weight m-chunks ride the initially-idle Act queue, so the
first real matmul starts ~3 us after the program body opens; slots 1-3
inputs follow dependency-free on the same Sync FIFO (the Tile scheduler
keeps same-queue dep-free DMAs in program order, so FIFO position itself
is the priority).  Outputs ride the Act queue; PSUM drains stay on DVE
(putting drains on ACT couples them to the 0.6-1.7 us output-trigger
instructions and stalls PSUM recycling; SWDGE outputs serialize on the
Q7).  A short PE warmup bridges the HAM cold window; slot 0's first rows
are chased in k-chunks sized >= 2 k-tiles (drain bandwidth, not DMA,
limits smaller accumulation groups); the last row of the last slot ends
with its smallest j-tile and per-j output stores to minimize the tail.
"""

import os

import numpy as np

P = 128
B = 8192
D_LAT = 1024
D_EMB = 1024
D = D_LAT + D_EMB  # 2048
E = 8
TOPK = 2
N_CORES = 8
KT = D // P  # 16
NSLOT = 4
KQ = KT // NSLOT  # k-tiles per quarter-job = 4
MT = D // P  # 16

N_WARM = 6

_cache = {}


def _ntff_shim():
    import sys
    import types

    if "antenv.axon_hooks" in sys.modules:
        return
    holder = [None]
    mod = types.ModuleType("antenv.axon_hooks")
    mod.set_axon_ntff_profile_hook = lambda h: holder.__setitem__(0, h)
    mod.get_axon_ntff_profile_hook = lambda: holder[0]
    sys.modules["antenv.axon_hooks"] = mod
    try:
        import antenv

        antenv.axon_hooks = mod
        from trn_agent_boot.trn_boot import _ntff_profile_via_ctypes

        mod.set_axon_ntff_profile_hook(
            _ntff_profile_via_ctypes("/opt/axon/libaxon_pjrt.so")
        )
    except Exception:
        pass


def _n_tiles(C):
    assert C >= 512
    k = (C - 256) // 512 if C % 512 else C // 512
    rem = C - 512 * k
    sizes = [512] * k
    if rem == 0:
        pass
    elif rem <= 512:
        sizes.append(rem)
    else:
        sizes.extend([rem - 256, 256])
    return sizes


def _build(S):
    """S: tuple of 4 slot column-capacities (descending)."""
    import concourse.mybir as mybir
    from concourse import bacc
    from concourse.bass import ds
    from concourse.tile import TileContext

    dt = mybir.dt.bfloat16
    f32 = mybir.dt.float32

    def tiles_of(C):
        sizes = _n_tiles(C)
        offs = [0] * len(sizes)
        for j in range(1, len(sizes)):
            offs[j] = offs[j - 1] + sizes[j - 1]
        return sizes, offs

    sl_sizes = []
    sl_offs = []
    for C in S:
        sz, of = tiles_of(C)
        sl_sizes.append(sz)
        sl_offs.append(of)

    nc = bacc.Bacc(
        "TRN2", target_bir_lowering=False, debug=False, num_devices=N_CORES
    )
    # per-slot weights w[ki, m, kl, o] and tokens t[ki, kl, c], partition-major
    w_dram = [
        nc.dram_tensor(f"w{i}", [P, MT, KQ, P], dt, kind="ExternalInput")
        for i in range(NSLOT)
    ]
    t_dram = [
        nc.dram_tensor(f"t{i}", [P, KQ, S[i]], dt, kind="ExternalInput")
        for i in range(NSLOT)
    ]
    out_dram = [
        nc.dram_tensor(f"out{i}", [MT, P, S[i]], dt, kind="ExternalOutput")
        for i in range(NSLOT)
    ]

    with TileContext(nc) as tc:
        with (
            tc.tile_pool(name="w", bufs=1) as w_pool,
            tc.tile_pool(name="tok", bufs=1) as tok_pool,
            tc.tile_pool(name="acc", bufs=1) as acc_pool,
            tc.tile_pool(name="orow", bufs=6) as orow_pool,
            tc.tile_pool(name="warm", bufs=1) as warm_pool,
            tc.tile_pool(name="ps", bufs=8, space="PSUM") as ps_pool,
        ):
            # ---- tiles ----
            # slot 0: fine-grained for the startup chase
            w0_tiles = [
                w_pool.tile([P, hi - lo, KQ, P], dt, tag=f"w0_{lo}",
                            name=f"w0_{lo}")
                for lo, hi in [(0, 1), (1, 2), (2, 4), (4, 16)]
            ]
            W0_GROUPS = [(0, 1), (1, 2), (2, 4), (4, 16)]
            t0a = tok_pool.tile([P, 1, 512], dt, tag="t0a", name="t0a")
            t0b = tok_pool.tile([P, 1, S[0] - 512], dt, tag="t0b", name="t0b")
            t0c = tok_pool.tile([P, 1, S[0]], dt, tag="t0c", name="t0c")
            t0d = tok_pool.tile([P, 2, S[0]], dt, tag="t0d", name="t0d")
            # slots 1..3: single-tile weights/tokens
            w_tiles = [None] + [
                w_pool.tile([P, MT, KQ, P], dt, tag=f"w{i}", name=f"w{i}")
                for i in range(1, NSLOT)
            ]
            t_tiles = [None] + [
                tok_pool.tile([P, KQ, S[i]], dt, tag=f"t{i}", name=f"t{i}")
                for i in range(1, NSLOT)
            ]

            # ---- PE warmup ----
            warm = warm_pool.tile([P, 512], dt)
            nc.gpsimd.memset(warm[:], 0)
            wps = ps_pool.tile([P, 512], f32, tag="ps")
            for i in range(N_WARM):
                nc.tensor.matmul(
                    wps, warm[:, :128], warm[:],
                    start=(i == 0), stop=(i == N_WARM - 1),
                )
            nc.vector.tensor_copy(warm[:], wps)

            # ---- input DMAs ----
            # Tokens ride the Sync queue in consumption order; the first
            # weight m-chunks ride the (initially idle) Act queue so neither
            # stream queues behind the other in the critical first ~10 us.
            nc.sync.dma_start(t0a[:], t_dram[0].ap()[:, ds(0, 1), ds(0, 512)])
            nc.scalar.dma_start(w0_tiles[0][:], w_dram[0].ap()[:, ds(0, 1)])
            nc.sync.dma_start(
                t0b[:], t_dram[0].ap()[:, ds(0, 1), ds(512, S[0] - 512)]
            )
            nc.scalar.dma_start(w0_tiles[1][:], w_dram[0].ap()[:, ds(1, 1)])
            nc.sync.dma_start(t0c[:], t_dram[0].ap()[:, ds(1, 1)])
            nc.scalar.dma_start(w0_tiles[2][:], w_dram[0].ap()[:, ds(2, 2)])
            nc.sync.dma_start(t0d[:], t_dram[0].ap()[:, ds(2, 2)])
            nc.sync.dma_start(w0_tiles[3][:], w_dram[0].ap()[:, ds(4, 12)])

            # slots 1..3 inputs: dependency-free on the same Sync queue —
            # the scheduler keeps same-queue dep-free DMAs in program order,
            # so FIFO position itself prioritizes the slot-0 stream.
            for slot in range(1, NSLOT):
                nc.sync.dma_start(w_tiles[slot][:], w_dram[slot].ap()[:])
                nc.sync.dma_start(t_tiles[slot][:], t_dram[slot].ap()[:])

            def lhs0(m, k):
                for gi, (lo, hi) in enumerate(W0_GROUPS):
                    if m < hi:
                        return w0_tiles[gi][:, m - lo, k, :]
                raise AssertionError

            def rhs0(k, j):
                sz, of = sl_sizes[0], sl_offs[0]
                if k == 0:
                    if j == 0:
                        return t0a[:, 0, :]
                    return t0b[:, 0, ds(of[j] - 512, sz[j])]
                if k == 1:
                    return t0c[:, 0, ds(of[j], sz[j])]
                return t0d[:, k - 2, ds(of[j], sz[j])]

            def out_halves(i):
                sizes, offs = sl_sizes[i], sl_offs[i]
                J = len(sizes)
                ja = 2 if J >= 3 else (1 if J >= 2 else 0)
                h_split = offs[ja] + sizes[ja]
                return ja, h_split

            # ---- slot 0 startup: m0..m3 chased in k-chunks ----
            chunks = {
                0: [(0, 1), (1, 2), (2, 4)],
                1: [(0, 2), (2, 4)],
                2: [(0, 2), (2, 4)],
                3: [(0, 2), (2, 4)],
            }
            emit = [(0, 0), (0, 1), (1, 0), (2, 0), (3, 0),
                    (0, 2), (1, 1), (2, 1), (3, 1)]
            J0 = len(sl_sizes[0])
            ja0, h0_split = out_halves(0)
            acc_tiles = {}
            orow_q = {}
            for m, qi in emit:
                klo, khi = chunks[m][qi]
                last = qi == len(chunks[m]) - 1
                for j in range(J0):
                    psf = ps_pool.tile([P, 512], f32, tag="ps")
                    pj = psf[:, : sl_sizes[0][j]]
                    for k in range(klo, khi):
                        nc.tensor.matmul(
                            pj, lhs0(m, k), rhs0(k, j),
                            start=(k == klo), stop=(k == khi - 1),
                        )
                    if qi == 0:
                        a_full = acc_pool.tile([P, 512], f32, tag=f"acc{m}_{j}")
                        a = a_full[:, : sl_sizes[0][j]]
                        acc_tiles[(m, j)] = a
                        nc.scalar.copy(a, pj)
                    elif not last:
                        a = acc_tiles[(m, j)]
                        nc.vector.tensor_add(a, a, pj)
                    else:
                        if m not in orow_q:
                            orow_q[m] = orow_pool.tile(
                                [P, S[0]], dt, tag="orow", name=f"orow_q{m}"
                            )
                        o = orow_q[m][:, ds(sl_offs[0][j], sl_sizes[0][j])]
                        nc.vector.tensor_add(o, acc_tiles[(m, j)], pj)
                        if j == ja0:
                            nc.scalar.dma_start(
                                out_dram[0].ap()[m][:, ds(0, h0_split)],
                                orow_q[m][:, ds(0, h0_split)],
                            )
                if last:
                    nc.scalar.dma_start(
                        out_dram[0].ap()[m][:, ds(h0_split, S[0] - h0_split)],
                        orow_q[m][:, ds(h0_split, S[0] - h0_split)],
                    )

            def steady_row(slot, m, lhs, rhs, nm, last_row=False):
                sizes, offs = sl_sizes[slot], sl_offs[slot]
                C = S[slot]
                J = len(sizes)
                ja, h_split = out_halves(slot)
                orow = orow_pool.tile([P, C], dt, tag="orow", name=nm)
                od = out_dram[slot]
                for j in range(J):
                    psf = ps_pool.tile([P, 512], f32, tag="ps")
                    pj = psf[:, : sizes[j]]
                    for k in range(KQ):
                        nc.tensor.matmul(
                            pj, lhs(m, k), rhs(k, j),
                            start=(k == 0), stop=(k == KQ - 1),
                        )
                    o = orow[:, ds(offs[j], sizes[j])]
                    nc.vector.tensor_copy(o, pj)
                    if last_row:
                        nc.scalar.dma_start(
                            od.ap()[m][:, ds(offs[j], sizes[j])], o
                        )
                    elif j == ja:
                        nc.scalar.dma_start(
                            od.ap()[m][:, ds(0, h_split)],
                            orow[:, ds(0, h_split)],
                        )
                if not last_row:
                    nc.scalar.dma_start(
                        od.ap()[m][:, ds(h_split, C - h_split)],
                        orow[:, ds(h_split, C - h_split)],
                    )

            def mk_lhs(slot):
                return lambda m, k: w_tiles[slot][:, m, k, :]

            def mk_rhs(slot):
                sizes, offs = sl_sizes[slot], sl_offs[slot]
                return lambda k, j: t_tiles[slot][
                    :, k, ds(offs[j], sizes[j])
                ]

            # slot 0 steady rows (m4..15); then slots 1, 2; slot 3 with its
            # last row trailing for the minimal tail
            for m in range(4, MT):
                steady_row(0, m, lhs0, rhs0, f"o0_{m}")
            for slot in (1, 2):
                lh, rh = mk_lhs(slot), mk_rhs(slot)
                for m in range(MT):
                    steady_row(slot, m, lh, rh, f"o{slot}_{m}")
            lh, rh = mk_lhs(3), mk_rhs(3)
            for m in range(MT - 1):
                steady_row(3, m, lh, rh, f"o3_{m}")
            steady_row(3, MT - 1, lh, rh, "o3_last", last_row=True)
    nc.compile()
    return nc


def _get_program(S):
    key = tuple(S)
    if key not in _cache:
        _cache[key] = _build(key)
    return _cache[key]


# ------------------------------------------------------------------- host ---


def kernel(x, y, W_experts, b_experts, W_gate, b_gate):
    import ml_dtypes

    bf16 = np.dtype(ml_dtypes.bfloat16)

    x = np.asarray(x, dtype=np.float32)
    y = np.asarray(y, dtype=np.float32)
    W_experts = np.asarray(W_experts, dtype=np.float32)
    b_experts = np.asarray(b_experts, dtype=np.float32)
    W_gate = np.asarray(W_gate, dtype=np.float32)
    b_gate = np.asarray(b_gate, dtype=np.float32)

    inp = np.concatenate([x, y], axis=1)  # [B, D]

    # ---- routing (host) ----
    logits = inp.astype(np.float64) @ W_gate.T.astype(np.float64) + b_gate
    order = np.argsort(-logits, axis=1, kind="stable")
    top2 = order[:, :TOPK]
    v = np.take_along_axis(logits, top2, axis=1)
    v = v - v.max(axis=1, keepdims=True)
    ev = np.exp(v)
    g = (ev / ev.sum(axis=1, keepdims=True)).astype(np.float32)

    counts = np.bincount(top2.ravel(), minlength=E)

    idx_list = []
    wgt_list = []
    for e in range(E):
        m0 = top2[:, 0] == e
        m1 = top2[:, 1] == e
        idx_e = np.concatenate([np.nonzero(m0)[0], np.nonzero(m1)[0]])
        w_e = np.concatenate([g[m0, 0], g[m1, 1]])
        idx_list.append(idx_e)
        wgt_list.append(w_e)

    # ---- slot assignment: slot s holds experts ranked (2s, 2s+1) ----
    by_size = np.argsort(-counts, kind="stable")
    S = tuple(
        max(512, int(counts[by_size[2 * s]])) for s in range(NSLOT)
    )
    # core c, slot s -> (expert by_size[2s + c//4], quarter c%4)
    core_jobs = []  # per core: list of (expert, quarter) per slot
    for c in range(N_CORES):
        jobs = []
        for s in range(NSLOT):
            e = int(by_size[2 * s + c // 4])
            jobs.append((e, c % 4))
        core_jobs.append(jobs)

    inp_bf = inp.astype(bf16)
    w_r = W_experts.reshape(E, MT, P, KT, P)

    def w_quarter(e, q):
        # [P(ki), MT, KQ, P(o)] bf16
        return np.ascontiguousarray(
            w_r[e][:, :, q * KQ : (q + 1) * KQ, :]
            .transpose(3, 0, 2, 1)
            .astype(bf16)
        )

    tok_cache = {}

    def tok_quarter(e, q, C):
        key = e
        if key not in tok_cache:
            tok_cache[key] = inp_bf[idx_list[e]].T.reshape(KT, P, -1)
        sel = tok_cache[key][q * KQ : (q + 1) * KQ].transpose(1, 0, 2)
        n_e = len(idx_list[e])
        out = np.zeros((P, KQ, C), dtype=bf16)
        out[:, :, :n_e] = sel
        return out

    in_maps = []
    for c in range(N_CORES):
        m = {}
        for s, (e, q) in enumerate(core_jobs[c]):
            m[f"w{s}"] = w_quarter(e, q)
            m[f"t{s}"] = tok_quarter(e, q, S[s])
        in_maps.append(m)

    # ---- device ----
    if os.environ.get("BASS_TRACE"):
        _ntff_shim()
    from concourse.bass_utils import run_bass_kernel_spmd

    nc = _get_program(S)
    res = None
    for attempt in range(3):
        try:
            res = run_bass_kernel_spmd(nc, in_maps, core_ids=list(range(N_CORES)))
            break
        except Exception:
            if attempt == 2:
                raise
            import time

            time.sleep(20 * (attempt + 1))
            try:
                import jax

                jax.clear_caches()
            except Exception:
                pass
    globals()["_last_res"] = res
    if res.exec_time_ns is not None:
        print(f"HW exec time: {res.exec_time_ns} ns")

    # ---- combine (host): sum 4 quarter partials, bias, gates, scatter ----
    part = {}  # (expert, quarter) -> [n_e, D] f32
    for c in range(N_CORES):
        for s, (e, q) in enumerate(core_jobs[c]):
            n_e = len(idx_list[e])
            part[(e, q)] = (
                res.results[c][f"out{s}"]
                .reshape(D, S[s])[:, :n_e]
                .T.astype(np.float32)
            )

    fused = np.zeros((B, D), dtype=np.float32)
    for e in range(E):
        n_e = len(idx_list[e])
        if n_e == 0:
            continue
        rows = part[(e, 0)] + part[(e, 1)] + part[(e, 2)] + part[(e, 3)]
        rows += b_experts[e]
        fused[idx_list[e]] += rows * wgt_list[e][:, None]
    return fused
